# revision 2
# baseline (speedup 1.0000x reference)
"""Trainium2 Bass kernel for nn_AdaBoostClassifier (8-core data-parallel).

Reference computation:
    logits = x @ W.T + b                      # [N, E]
    preds  = round(sigmoid(logits))           # {0,1} == 1[logit > 0]
    acc    = sum_e trunc(alpha_e) * preds_e   # integer-valued
    out    = sign(acc)

Math used here: with t = trunc(alphas), s_e = Sign(logit_e) in {-1,0,1}:
    acc = (sum_e t_e * s_e + sum_e t_e) / 2   (exact; s=0 measure-zero)
so out = Sign(dot(t, s) + T) with T = sum(t). Columns with t_e == 0
contribute nothing, so only those estimators are computed (selected on
host at runtime — valid for any input values).

Device pipeline per 512-sample block:
  3-pass mixed-precision matmul (xh fp16 + xl bf16 splits of x and W;
  products are exact in fp32 PSUM, total precision ~2^-20 — empirically
  bit-identical sign outputs vs fp32) -> ACT Sign(psum + b) -> PE matvec
  with t -> ACT Sign(acc + T) -> DMA out.
"""

import math

import numpy as np
import ml_dtypes

import concourse.bass as bass  # noqa: F401  (registers bass types)
import concourse.tile as tile
from concourse import bacc, mybir
from concourse.bass_utils import run_bass_kernel_spmd

BF16 = ml_dtypes.bfloat16

N_CORES = 8
N_FULL = 131072
F_DIM = 512
NS = N_FULL // N_CORES          # samples per core
BLK = 512                       # samples per psum block (one PSUM bank)
SUPER = 2048                    # samples per DMA super-block
N_SUPER = NS // SUPER
SUBS = SUPER // BLK
KC = F_DIM // 128               # contraction chunks

_program_cache: dict[int, object] = {}


def _build(n_etiles: int):
    """Build the 8-core SPMD program for n_etiles 128-wide estimator tiles."""
    nc = bacc.Bacc("TRN2", target_bir_lowering=False, debug=False)

    d_xh = nc.dram_tensor("xh", [F_DIM, NS], mybir.dt.float16, kind="ExternalInput")
    d_xl = nc.dram_tensor("xl", [F_DIM, NS], mybir.dt.bfloat16, kind="ExternalInput")
    d_wh = nc.dram_tensor(
        "wh", [n_etiles, F_DIM, 128], mybir.dt.float16, kind="ExternalInput"
    )
    d_wl = nc.dram_tensor(
        "wl", [n_etiles, F_DIM, 128], mybir.dt.bfloat16, kind="ExternalInput"
    )
    d_bv = nc.dram_tensor("bv", [n_etiles, 128, 1], mybir.dt.float32, kind="ExternalInput")
    d_tv = nc.dram_tensor("tv", [n_etiles, 128, 1], mybir.dt.bfloat16, kind="ExternalInput")
    d_tt = nc.dram_tensor("tt", [1, 1], mybir.dt.float32, kind="ExternalInput")
    d_out = nc.dram_tensor("out", [NS], mybir.dt.float32, kind="ExternalOutput")

    # f-major views of x planes: (k p) n -> p k n, so chunk k = features
    # [128k, 128k+128) on partitions.
    xh_v = d_xh.ap().rearrange("(k p) n -> p k n", p=128)
    xl_v = d_xl.ap().rearrange("(k p) n -> p k n", p=128)
    out_v = d_out.ap().rearrange("(s n) -> s n", n=SUPER)

    with tile.TileContext(nc) as tc:
        with (
            tc.tile_pool(name="singles", bufs=1) as singles,
            tc.tile_pool(name="xbuf", bufs=2) as xbuf,
            tc.tile_pool(name="sbuf", bufs=3) as spool,
            tc.tile_pool(name="obuf", bufs=2) as obuf,
            tc.tile_pool(name="pslog", bufs=2, space="PSUM") as pslog,
            tc.tile_pool(name="psacc", bufs=2, space="PSUM") as psacc,
        ):
            # --- weights / per-estimator constants (loaded once) ---
            wh_t = {}
            wl_t = {}
            bv_t = {}
            tv_t = {}
            for j in range(n_etiles):
                for k in range(KC):
                    wh_t[j, k] = singles.tile([128, 128], mybir.dt.float16,
                                              tag=f"wh{j}_{k}", name=f"wh{j}_{k}")
                    nc.sync.dma_start(
                        out=wh_t[j, k],
                        in_=d_wh.ap()[j, k * 128:(k + 1) * 128, :],
                    )
                    wl_t[j, k] = singles.tile([128, 128], mybir.dt.bfloat16,
                                              tag=f"wl{j}_{k}", name=f"wl{j}_{k}")
                    nc.sync.dma_start(
                        out=wl_t[j, k],
                        in_=d_wl.ap()[j, k * 128:(k + 1) * 128, :],
                    )
                bv_t[j] = singles.tile([128, 1], mybir.dt.float32, tag=f"bv{j}", name=f"bv{j}")
                nc.sync.dma_start(out=bv_t[j], in_=d_bv.ap()[j])
                tv_t[j] = singles.tile([128, 1], mybir.dt.bfloat16, tag=f"tv{j}", name=f"tv{j}")
                nc.sync.dma_start(out=tv_t[j], in_=d_tv.ap()[j])
            tt_t = singles.tile([1, 1], mybir.dt.float32, tag="tt")
            nc.sync.dma_start(out=tt_t, in_=d_tt.ap())

            # --- main loop ---
            for sb in range(N_SUPER):
                n0 = sb * SUPER
                xh_sb = xbuf.tile([128, KC, SUPER], mybir.dt.float16, tag="xh")
                nc.sync.dma_start(out=xh_sb, in_=xh_v[:, :, n0:n0 + SUPER])
                xl_sb = xbuf.tile([128, KC, SUPER], mybir.dt.bfloat16, tag="xl")
                nc.sync.dma_start(out=xl_sb, in_=xl_v[:, :, n0:n0 + SUPER])

                out_sb = obuf.tile([1, SUPER], mybir.dt.float32, tag="osb")

                for s in range(SUBS):
                    ns = slice(s * BLK, (s + 1) * BLK)
                    acc = psacc.tile([1, BLK], mybir.dt.float32, tag="acc")
                    for j in range(n_etiles):
                        logits = pslog.tile([128, BLK], mybir.dt.float32, tag="lg")
                        mm = 0
                        for k in range(KC):
                            nc.tensor.matmul(
                                logits, wh_t[j, k], xh_sb[:, k, ns],
                                start=(mm == 0), stop=False,
                            )
                            mm += 1
                            nc.tensor.matmul(
                                logits, wl_t[j, k], xh_sb[:, k, ns],
                                start=False, stop=False,
                            )
                            mm += 1
                            nc.tensor.matmul(
                                logits, wh_t[j, k], xl_sb[:, k, ns],
                                start=False, stop=(k == KC - 1),
                            )
                            mm += 1
                        s_t = spool.tile([128, BLK], mybir.dt.bfloat16, tag="sg")
                        nc.scalar.activation(
                            out=s_t, in_=logits,
                            func=mybir.ActivationFunctionType.Sign,
                            bias=bv_t[j],
                        )
                        nc.tensor.matmul(
                            acc, tv_t[j], s_t,
                            start=(j == 0), stop=(j == n_etiles - 1),
                        )
                    nc.scalar.activation(
                        out=out_sb[0:1, ns], in_=acc,
                        func=mybir.ActivationFunctionType.Sign,
                        bias=tt_t,
                    )
                nc.sync.dma_start(out=out_v[sb:sb + 1, :], in_=out_sb)

    nc.compile()
    return nc


def _prep_inputs(x, W, b, alphas):
    """Host-side prep: estimator selection, transposes, hi/lo splits."""
    t_full = np.trunc(alphas.astype(np.float32)).astype(np.float32)
    T = float(t_full.sum())
    nz = np.flatnonzero(t_full)
    n_etiles = max(1, math.ceil(len(nz) / 128))
    e_pad = n_etiles * 128

    W_sel = np.zeros((e_pad, F_DIM), np.float32)
    b_sel = np.zeros((e_pad,), np.float32)
    t_sel = np.zeros((e_pad,), np.float32)
    if len(nz):
        W_sel[: len(nz)] = W[nz]
        b_sel[: len(nz)] = b[nz]
        t_sel[: len(nz)] = t_full[nz]

    # [n_etiles, F, 128] stationary layout (partition = feature)
    w_fe = W_sel.T.reshape(F_DIM, n_etiles, 128).transpose(1, 0, 2)
    wh = w_fe.astype(np.float16)
    wl = (w_fe - wh.astype(np.float32)).astype(BF16)

    xT = np.ascontiguousarray(x.T.astype(np.float32))  # [F, N]
    xh = xT.astype(np.float16)
    xl = (xT - xh.astype(np.float32)).astype(BF16)

    bv = np.ascontiguousarray(b_sel.reshape(n_etiles, 128, 1))
    tv = np.ascontiguousarray(t_sel.reshape(n_etiles, 128, 1)).astype(BF16)
    tt = np.array([[T]], np.float32)

    in_maps = []
    for c in range(N_CORES):
        sl = slice(c * NS, (c + 1) * NS)
        in_maps.append({
            "xh": np.ascontiguousarray(xh[:, sl]),
            "xl": np.ascontiguousarray(xl[:, sl]),
            "wh": wh, "wl": wl, "bv": bv, "tv": tv, "tt": tt,
        })
    return n_etiles, in_maps


def kernel(x, W, b, alphas, _trace=False, _trace_kwargs=None):
    n_etiles, in_maps = _prep_inputs(
        np.asarray(x), np.asarray(W), np.asarray(b), np.asarray(alphas)
    )
    nc = _program_cache.get(n_etiles)
    if nc is None:
        nc = _build(n_etiles)
        _program_cache[n_etiles] = nc

    kwargs = {}
    if _trace:
        kwargs["trace"] = True
        kwargs.update(_trace_kwargs or {})
    res = run_bass_kernel_spmd(nc, in_maps, core_ids=list(range(N_CORES)), **kwargs)
    out = np.concatenate([res.results[c]["out"] for c in range(N_CORES)])
    if _trace:
        kernel.last_results = res
    return out.astype(np.float32)


# revision 4
# speedup vs baseline: 1.1401x; 1.1401x over previous
"""Trainium2 Bass kernel for nn_AdaBoostClassifier (8-core data-parallel).

Reference computation:
    logits = x @ W.T + b                      # [N, E]
    preds  = round(sigmoid(logits))           # {0,1} == 1[logit > 0]
    acc    = sum_e trunc(alpha_e) * preds_e   # integer-valued
    out    = sign(acc)

Math used here: with t = trunc(alphas), s_e = Sign(logit_e) in {-1,0,1}:
    acc = (sum_e t_e * s_e + sum_e t_e) / 2   (exact; s=0 measure-zero)
so out = Sign(dot(t, s) + T) with T = sum(t). Columns with t_e == 0
contribute nothing, so only those estimators are computed (selected on
host at runtime — valid for any input values).

Device pipeline per 512-sample block:
  3-pass mixed-precision matmul (xh fp16 + xl bf16 splits of x and W;
  products are exact in fp32 PSUM, total precision ~2^-20 — empirically
  bit-identical sign outputs vs fp32) -> ACT Sign(psum + b) -> PE matvec
  with t -> ACT Sign(acc + T) -> DMA out.
"""

import math

import numpy as np
import ml_dtypes

import concourse.bass as bass  # noqa: F401  (registers bass types)
import concourse.tile as tile
from concourse import bacc, mybir
from concourse.bass_utils import run_bass_kernel_spmd

BF16 = ml_dtypes.bfloat16

N_CORES = 8
N_FULL = 131072
F_DIM = 512
NS = N_FULL // N_CORES          # samples per core
BLK = 512                       # samples per psum block (one PSUM bank)
SUPER = 2048                    # samples per DMA super-block
N_SUPER = NS // SUPER
SUBS = SUPER // BLK
KC = F_DIM // 128               # contraction chunks

_program_cache: dict[int, object] = {}


def _build(n_etiles: int):
    """Build the 8-core SPMD program for n_etiles 128-wide estimator tiles."""
    nc = bacc.Bacc("TRN2", target_bir_lowering=False, debug=False)

    d_xh = nc.dram_tensor("xh", [F_DIM, NS], mybir.dt.float16, kind="ExternalInput")
    d_xl = nc.dram_tensor("xl", [F_DIM, NS], mybir.dt.bfloat16, kind="ExternalInput")
    d_wh = nc.dram_tensor(
        "wh", [n_etiles, F_DIM, 128], mybir.dt.float16, kind="ExternalInput"
    )
    d_wl = nc.dram_tensor(
        "wl", [n_etiles, F_DIM, 128], mybir.dt.bfloat16, kind="ExternalInput"
    )
    d_bv = nc.dram_tensor("bv", [n_etiles, 128, 1], mybir.dt.float32, kind="ExternalInput")
    d_tv = nc.dram_tensor("tv", [n_etiles, 128, 1], mybir.dt.bfloat16, kind="ExternalInput")
    d_tt = nc.dram_tensor("tt", [1, 1], mybir.dt.float32, kind="ExternalInput")
    d_out = nc.dram_tensor("out", [NS], mybir.dt.float32, kind="ExternalOutput")

    # f-major views of x planes: (k p) n -> p k n, so chunk k = features
    # [128k, 128k+128) on partitions.
    xh_v = d_xh.ap().rearrange("(k p) n -> p k n", p=128)
    xl_v = d_xl.ap().rearrange("(k p) n -> p k n", p=128)
    out_v = d_out.ap().rearrange("(s n) -> s n", n=SUPER)

    with tile.TileContext(nc) as tc:
        with (
            tc.tile_pool(name="singles", bufs=1) as singles,
            tc.tile_pool(name="xbuf", bufs=4) as xbuf,
            tc.tile_pool(name="sbuf", bufs=3) as spool,
            tc.tile_pool(name="obuf", bufs=2) as obuf,
            tc.tile_pool(name="pslog", bufs=2, space="PSUM") as pslog,
            tc.tile_pool(name="psacc", bufs=2, space="PSUM") as psacc,
        ):
            # --- weights / per-estimator constants: few batched DMAs on the
            # ACT HWDGE ring so the SP ring streams x from cycle 0 ---
            wh_t = singles.tile([128, n_etiles, KC, 128], mybir.dt.float16, tag="wh")
            nc.scalar.dma_start(
                out=wh_t, in_=d_wh.ap().rearrange("j (k p) e -> p j k e", p=128)
            )
            wl_t = singles.tile([128, n_etiles, KC, 128], mybir.dt.bfloat16, tag="wl")
            nc.scalar.dma_start(
                out=wl_t, in_=d_wl.ap().rearrange("j (k p) e -> p j k e", p=128)
            )
            bv_t = singles.tile([128, n_etiles], mybir.dt.float32, tag="bv")
            nc.scalar.dma_start(
                out=bv_t, in_=d_bv.ap().rearrange("j p one -> p (j one)")
            )
            tv_t = singles.tile([128, n_etiles], mybir.dt.bfloat16, tag="tv")
            nc.scalar.dma_start(
                out=tv_t, in_=d_tv.ap().rearrange("j p one -> p (j one)")
            )
            tt_t = singles.tile([1, 1], mybir.dt.float32, tag="tt")
            nc.scalar.dma_start(out=tt_t, in_=d_tt.ap())

            # --- main loop ---
            for sb in range(N_SUPER):
                n0 = sb * SUPER
                xh_sb = xbuf.tile([128, KC, SUPER], mybir.dt.float16, tag="xh")
                nc.sync.dma_start(out=xh_sb, in_=xh_v[:, :, n0:n0 + SUPER])
                xl_sb = xbuf.tile([128, KC, SUPER], mybir.dt.bfloat16, tag="xl")
                nc.sync.dma_start(out=xl_sb, in_=xl_v[:, :, n0:n0 + SUPER])

                out_sb = obuf.tile([1, SUPER], mybir.dt.float32, tag="osb")

                for s in range(SUBS):
                    ns = slice(s * BLK, (s + 1) * BLK)
                    acc = psacc.tile([1, BLK], mybir.dt.float32, tag="acc")
                    for j in range(n_etiles):
                        logits = pslog.tile([128, BLK], mybir.dt.float32, tag="lg")
                        mm = 0
                        for k in range(KC):
                            nc.tensor.matmul(
                                logits, wh_t[:, j, k, :], xh_sb[:, k, ns],
                                start=(mm == 0), stop=False,
                            )
                            mm += 1
                            nc.tensor.matmul(
                                logits, wl_t[:, j, k, :], xh_sb[:, k, ns],
                                start=False, stop=False,
                            )
                            mm += 1
                            nc.tensor.matmul(
                                logits, wh_t[:, j, k, :], xl_sb[:, k, ns],
                                start=False, stop=(k == KC - 1),
                            )
                            mm += 1
                        s_t = spool.tile([128, BLK], mybir.dt.bfloat16, tag="sg")
                        nc.scalar.activation(
                            out=s_t, in_=logits,
                            func=mybir.ActivationFunctionType.Sign,
                            bias=bv_t[:, j:j + 1],
                        )
                        nc.tensor.matmul(
                            acc, tv_t[:, j:j + 1], s_t,
                            start=(j == 0), stop=(j == n_etiles - 1),
                        )
                    nc.scalar.activation(
                        out=out_sb[0:1, ns], in_=acc,
                        func=mybir.ActivationFunctionType.Sign,
                        bias=tt_t,
                    )
                nc.scalar.dma_start(out=out_v[sb:sb + 1, :], in_=out_sb)

    nc.compile()
    return nc


def _prep_inputs(x, W, b, alphas):
    """Host-side prep: estimator selection, transposes, hi/lo splits."""
    t_full = np.trunc(alphas.astype(np.float32)).astype(np.float32)
    T = float(t_full.sum())
    nz = np.flatnonzero(t_full)
    n_etiles = max(1, math.ceil(len(nz) / 128))
    e_pad = n_etiles * 128

    W_sel = np.zeros((e_pad, F_DIM), np.float32)
    b_sel = np.zeros((e_pad,), np.float32)
    t_sel = np.zeros((e_pad,), np.float32)
    if len(nz):
        W_sel[: len(nz)] = W[nz]
        b_sel[: len(nz)] = b[nz]
        t_sel[: len(nz)] = t_full[nz]

    # [n_etiles, F, 128] stationary layout (partition = feature)
    w_fe = W_sel.T.reshape(F_DIM, n_etiles, 128).transpose(1, 0, 2)
    wh = w_fe.astype(np.float16)
    wl = (w_fe - wh.astype(np.float32)).astype(BF16)

    xT = np.ascontiguousarray(x.T.astype(np.float32))  # [F, N]
    xh = xT.astype(np.float16)
    xl = (xT - xh.astype(np.float32)).astype(BF16)

    bv = np.ascontiguousarray(b_sel.reshape(n_etiles, 128, 1))
    tv = np.ascontiguousarray(t_sel.reshape(n_etiles, 128, 1)).astype(BF16)
    tt = np.array([[T]], np.float32)

    in_maps = []
    for c in range(N_CORES):
        sl = slice(c * NS, (c + 1) * NS)
        in_maps.append({
            "xh": np.ascontiguousarray(xh[:, sl]),
            "xl": np.ascontiguousarray(xl[:, sl]),
            "wh": wh, "wl": wl, "bv": bv, "tv": tv, "tt": tt,
        })
    return n_etiles, in_maps


def kernel(x, W, b, alphas, _trace=False, _trace_kwargs=None):
    n_etiles, in_maps = _prep_inputs(
        np.asarray(x), np.asarray(W), np.asarray(b), np.asarray(alphas)
    )
    nc = _program_cache.get(n_etiles)
    if nc is None:
        nc = _build(n_etiles)
        _program_cache[n_etiles] = nc

    kwargs = {}
    if _trace:
        kwargs["trace"] = True
        kwargs.update(_trace_kwargs or {})
    res = run_bass_kernel_spmd(nc, in_maps, core_ids=list(range(N_CORES)), **kwargs)
    out = np.concatenate([res.results[c]["out"] for c in range(N_CORES)])
    if _trace:
        kernel.last_results = res
    return out.astype(np.float32)


# revision 5
# speedup vs baseline: 1.1868x; 1.0410x over previous
"""Trainium2 Bass kernel for nn_AdaBoostClassifier (8-core data-parallel).

Reference computation:
    logits = x @ W.T + b                      # [N, E]
    preds  = round(sigmoid(logits))           # {0,1} == 1[logit > 0]
    acc    = sum_e trunc(alpha_e) * preds_e   # integer-valued
    out    = sign(acc)

Math used here: with t = trunc(alphas), s_e = Sign(logit_e) in {-1,0,1}:
    acc = (sum_e t_e * s_e + sum_e t_e) / 2   (exact; s=0 measure-zero)
so out = Sign(dot(t, s) + T) with T = sum(t). Columns with t_e == 0
contribute nothing, so only those estimators are computed (selected on
host at runtime — valid for any input values).

Device pipeline per 512-sample block:
  3-pass mixed-precision matmul (xh fp16 + xl bf16 splits of x and W;
  products are exact in fp32 PSUM, total precision ~2^-20 — empirically
  bit-identical sign outputs vs fp32) -> ACT Sign(psum + b) -> PE matvec
  with t -> ACT Sign(acc + T) -> DMA out.
"""

import math

import numpy as np
import ml_dtypes

import concourse.bass as bass  # noqa: F401  (registers bass types)
import concourse.tile as tile
from concourse import bacc, mybir
from concourse.bass_utils import run_bass_kernel_spmd

BF16 = ml_dtypes.bfloat16

N_CORES = 8
N_FULL = 131072
F_DIM = 512
NS = N_FULL // N_CORES          # samples per core
BLK = 512                       # samples per psum block (one PSUM bank)
SUPER = 1024                    # samples per DMA super-block
N_SUPER = NS // SUPER
SUBS = SUPER // BLK
KC = F_DIM // 128               # contraction chunks

_program_cache: dict[int, object] = {}


def _build(n_etiles: int):
    """Build the 8-core SPMD program for n_etiles 128-wide estimator tiles."""
    nc = bacc.Bacc("TRN2", target_bir_lowering=False, debug=False)

    d_xh = nc.dram_tensor("xh", [F_DIM, NS], mybir.dt.float16, kind="ExternalInput")
    d_xl = nc.dram_tensor("xl", [F_DIM, NS], mybir.dt.bfloat16, kind="ExternalInput")
    d_wh = nc.dram_tensor(
        "wh", [n_etiles, F_DIM, 128], mybir.dt.float16, kind="ExternalInput"
    )
    d_wl = nc.dram_tensor(
        "wl", [n_etiles, F_DIM, 128], mybir.dt.bfloat16, kind="ExternalInput"
    )
    d_bv = nc.dram_tensor("bv", [n_etiles, 128, 1], mybir.dt.float32, kind="ExternalInput")
    d_tv = nc.dram_tensor("tv", [n_etiles, 128, 1], mybir.dt.bfloat16, kind="ExternalInput")
    d_tt = nc.dram_tensor("tt", [1, 1], mybir.dt.float32, kind="ExternalInput")
    d_out = nc.dram_tensor("out", [NS], mybir.dt.float32, kind="ExternalOutput")

    # f-major views of x planes: (k p) n -> p k n, so chunk k = features
    # [128k, 128k+128) on partitions.
    xh_v = d_xh.ap().rearrange("(k p) n -> p k n", p=128)
    xl_v = d_xl.ap().rearrange("(k p) n -> p k n", p=128)
    out_v = d_out.ap().rearrange("(s n) -> s n", n=SUPER)

    with tile.TileContext(nc) as tc:
        with (
            tc.tile_pool(name="singles", bufs=1) as singles,
            tc.tile_pool(name="xbuf", bufs=6) as xbuf,
            tc.tile_pool(name="sbuf", bufs=3) as spool,
            tc.tile_pool(name="obuf", bufs=2) as obuf,
            tc.tile_pool(name="pslog", bufs=2, space="PSUM") as pslog,
            tc.tile_pool(name="psacc", bufs=2, space="PSUM") as psacc,
        ):
            # --- weights / per-estimator constants: few batched DMAs on the
            # ACT HWDGE ring so the SP ring streams x from cycle 0 ---
            wh_t = singles.tile([128, n_etiles, KC, 128], mybir.dt.float16, tag="wh")
            nc.scalar.dma_start(
                out=wh_t, in_=d_wh.ap().rearrange("j (k p) e -> p j k e", p=128)
            )
            wl_t = singles.tile([128, n_etiles, KC, 128], mybir.dt.bfloat16, tag="wl")
            nc.scalar.dma_start(
                out=wl_t, in_=d_wl.ap().rearrange("j (k p) e -> p j k e", p=128)
            )
            bv_t = singles.tile([128, n_etiles], mybir.dt.float32, tag="bv")
            nc.scalar.dma_start(
                out=bv_t, in_=d_bv.ap().rearrange("j p one -> p (j one)")
            )
            tv_t = singles.tile([128, n_etiles], mybir.dt.bfloat16, tag="tv")
            nc.scalar.dma_start(
                out=tv_t, in_=d_tv.ap().rearrange("j p one -> p (j one)")
            )
            tt_t = singles.tile([1, 1], mybir.dt.float32, tag="tt")
            nc.scalar.dma_start(out=tt_t, in_=d_tt.ap())

            # --- main loop ---
            for sb in range(N_SUPER):
                n0 = sb * SUPER
                xh_sb = xbuf.tile([128, KC, SUPER], mybir.dt.float16, tag="xh")
                nc.sync.dma_start(out=xh_sb, in_=xh_v[:, :, n0:n0 + SUPER])
                xl_sb = xbuf.tile([128, KC, SUPER], mybir.dt.bfloat16, tag="xl")
                nc.sync.dma_start(out=xl_sb, in_=xl_v[:, :, n0:n0 + SUPER])

                out_sb = obuf.tile([1, SUPER], mybir.dt.float32, tag="osb")

                for s in range(SUBS):
                    ns = slice(s * BLK, (s + 1) * BLK)
                    acc = psacc.tile([1, BLK], mybir.dt.float32, tag="acc")
                    for j in range(n_etiles):
                        logits = pslog.tile([128, BLK], mybir.dt.float32, tag="lg")
                        # xh-dependent passes first so the PE can start as
                        # soon as xh lands, while xl is still streaming in.
                        passes = [(wh_t, xh_sb), (wl_t, xh_sb), (wh_t, xl_sb)]
                        mm = 0
                        for w_t, x_sb in passes:
                            for k in range(KC):
                                nc.tensor.matmul(
                                    logits, w_t[:, j, k, :], x_sb[:, k, ns],
                                    start=(mm == 0), stop=(mm == 3 * KC - 1),
                                )
                                mm += 1
                        s_t = spool.tile([128, BLK], mybir.dt.bfloat16, tag="sg")
                        nc.scalar.activation(
                            out=s_t, in_=logits,
                            func=mybir.ActivationFunctionType.Sign,
                            bias=bv_t[:, j:j + 1],
                        )
                        nc.tensor.matmul(
                            acc, tv_t[:, j:j + 1], s_t,
                            start=(j == 0), stop=(j == n_etiles - 1),
                        )
                    nc.scalar.activation(
                        out=out_sb[0:1, ns], in_=acc,
                        func=mybir.ActivationFunctionType.Sign,
                        bias=tt_t,
                    )
                nc.scalar.dma_start(out=out_v[sb:sb + 1, :], in_=out_sb)

    nc.compile()
    return nc


def _prep_inputs(x, W, b, alphas):
    """Host-side prep: estimator selection, transposes, hi/lo splits."""
    t_full = np.trunc(alphas.astype(np.float32)).astype(np.float32)
    T = float(t_full.sum())
    nz = np.flatnonzero(t_full)
    n_etiles = max(1, math.ceil(len(nz) / 128))
    e_pad = n_etiles * 128

    W_sel = np.zeros((e_pad, F_DIM), np.float32)
    b_sel = np.zeros((e_pad,), np.float32)
    t_sel = np.zeros((e_pad,), np.float32)
    if len(nz):
        W_sel[: len(nz)] = W[nz]
        b_sel[: len(nz)] = b[nz]
        t_sel[: len(nz)] = t_full[nz]

    # [n_etiles, F, 128] stationary layout (partition = feature)
    w_fe = W_sel.T.reshape(F_DIM, n_etiles, 128).transpose(1, 0, 2)
    wh = w_fe.astype(np.float16)
    wl = (w_fe - wh.astype(np.float32)).astype(BF16)

    xT = np.ascontiguousarray(x.T.astype(np.float32))  # [F, N]
    xh = xT.astype(np.float16)
    xl = (xT - xh.astype(np.float32)).astype(BF16)

    bv = np.ascontiguousarray(b_sel.reshape(n_etiles, 128, 1))
    tv = np.ascontiguousarray(t_sel.reshape(n_etiles, 128, 1)).astype(BF16)
    tt = np.array([[T]], np.float32)

    in_maps = []
    for c in range(N_CORES):
        sl = slice(c * NS, (c + 1) * NS)
        in_maps.append({
            "xh": np.ascontiguousarray(xh[:, sl]),
            "xl": np.ascontiguousarray(xl[:, sl]),
            "wh": wh, "wl": wl, "bv": bv, "tv": tv, "tt": tt,
        })
    return n_etiles, in_maps


def kernel(x, W, b, alphas, _trace=False, _trace_kwargs=None):
    n_etiles, in_maps = _prep_inputs(
        np.asarray(x), np.asarray(W), np.asarray(b), np.asarray(alphas)
    )
    nc = _program_cache.get(n_etiles)
    if nc is None:
        nc = _build(n_etiles)
        _program_cache[n_etiles] = nc

    kwargs = {}
    if _trace:
        kwargs["trace"] = True
        kwargs.update(_trace_kwargs or {})
    res = run_bass_kernel_spmd(nc, in_maps, core_ids=list(range(N_CORES)), **kwargs)
    out = np.concatenate([res.results[c]["out"] for c in range(N_CORES)])
    if _trace:
        kernel.last_results = res
    return out.astype(np.float32)
